# revision 5
# baseline (speedup 1.0000x reference)
"""Nearest-E8-lattice quantizer (CachedE8Quantizer) as a Bass/Tile kernel on 8 trn2 cores.

Input x: [8388608, 8] fp32. Output: nearest point of E8 = D8 u (D8 + 1/2).

Sharding: data-parallel over the points dim, 1/8 per core (no comms).

Algorithm (per 8-vector x), branch0 = D8, branch1 = D8 + 1/2:
  r0 = round(x) (RNE), d0 = x - r0 (exact), a = |d0|
  The two nearest half-grid points per coord are r0 and r1h = r0 + 0.5*s
  (s = sign(d0)), with |d1| = 0.5 - a. Hence:
    S2_1      = S2_0 - sum(a) + 2
    max|d1|   = 0.5 - min(a);  argmax|d1| = argmin(a)
    parity1   = parity0 XOR parity(#negative d0)   (since f1 = r0 - 1{d0<0})
  Candidate distances with the parity-repair term (flip the worst coord):
    D0 = S2 + p0*(1 - 2*max(a));  D1 = S2 - sum(a) + 2 + p1*(2*min(a))
    c = D0 <= D1  <=>  sum(a) - 2*p1*mn <= 2*p0*mx - p0 + 2
  Output:  y = r0 + s*(beta' + gamma*e),  gamma = 2c-1, beta' = 0.5*(1-c),
    e = 1{a == v*}, v* = gated flip target (mx if c&p0, mn if ~c&p1, else -1).
  s*(...) is applied in bit domain: sigma = (d0 & 0x80000000) XOR gamma.

Per-core per-tile engine split (tf points/partition, FE = 8*tf elems):
  DVE: r0 (TS 2x), 4-5 segmented reduces (1x), gate is_equal, sigma, ts, smalls
  ACT: a = Abs(d0), p0 = Square(ps), beta', PSUM->SBUF y copy
  GPSIMD: d0 = x - r0, min-reduce, q = e - beta'
  PE:  y = I*r0 + I*ts accumulated in PSUM
"""

import numpy as np

from concourse import bacc
import concourse.mybir as mybir
from concourse.alu_op_type import AluOpType as op
from concourse.bass_utils import run_bass_kernel_spmd
from concourse.tile import TileContext

N_POINTS = 8388608
N_CORES = 8
SHARD = N_POINTS // N_CORES  # 1048576 points per core

MAGIC = 12582912.0  # 1.5 * 2**23: (x + MAGIC) - MAGIC == round-half-even(x)
F32 = mybir.dt.float32
I32 = mybir.dt.int32
U32 = mybir.dt.uint32
X = mybir.AxisListType.X
CP = mybir.ActivationFunctionType.Copy

ENGINES = {
    "d0": "gpsimd",      # x - r0
    "mn": "vector",      # min-reduce engine
    "q": "gpsimd",       # q = e - beta'_b
    "e": "vector",       # gate equality
    "pe_y": True,        # final add on PE (else DVE)
}


def _stt_u32(eng, out, in0, scalar_int, in1, op0, op1):
    """scalar_tensor_tensor with a uint32 immediate."""
    return eng.add_instruction(
        mybir.InstTensorScalarPtr(
            name=eng.bass.get_next_instruction_name(),
            is_scalar_tensor_tensor=True,
            op0=op0,
            op1=op1,
            ins=[
                eng.lower_ap(in0),
                mybir.ImmediateValue(dtype=U32, value=scalar_int),
                eng.lower_ap(in1),
            ],
            outs=[eng.lower_ap(out)],
        )
    )


def _ts_u32_single(eng, out, in0, scalar_int, op0):
    """tensor_scalar with a uint32 immediate (single op)."""
    return eng.add_instruction(
        mybir.InstTensorScalarPtr(
            name=eng.bass.get_next_instruction_name(),
            op0=op0,
            ins=[
                eng.lower_ap(in0),
                mybir.ImmediateValue(dtype=U32, value=scalar_int),
            ],
            outs=[eng.lower_ap(out)],
        )
    )


def _emit_tile(nc, pools, xd, yd, t, tf):
    E = lambda k: getattr(nc, ENGINES[k])
    P = 128
    pts = P * tf
    FE = tf * 8
    stream, work, small = pools[:3]

    s0 = t * pts
    x_rows = xd[s0 : s0 + pts, :].rearrange("(p f) c -> p (f c)", p=P)
    y_rows = yd[s0 : s0 + pts, :].rearrange("(p f) c -> p (f c)", p=P)

    xt = stream.tile([P, FE], F32, tag="xt")
    nc.sync.dma_start(out=xt[:], in_=x_rows)

    # r0 = round(x)  (RNE via magic number), DVE TS 2x
    rr = work.tile([P, FE], F32, tag="rr")
    nc.vector.tensor_scalar(rr[:], xt[:], MAGIC, MAGIC, op0=op.add, op1=op.subtract)

    # d0 = x - r0 (exact)
    dd = work.tile([P, FE], F32, tag="dd")
    E("d0").tensor_tensor(dd[:], xt[:], rr[:], op.subtract)
    dd_u3 = dd[:].bitcast(U32).rearrange("p (t c) -> p t c", c=8)

    # a = |d0| on ACT
    aa = work.tile([P, FE], F32, tag="aa")
    nc.scalar.activation(aa[:], dd[:], mybir.ActivationFunctionType.Abs)
    aa3 = aa[:].rearrange("p (t c) -> p t c", c=8)
    rr3 = rr[:].rearrange("p (t c) -> p t c", c=8)

    # ---- segmented reduces -> per-point stats [P, tf] ----
    NS = 15
    arena = small.tile([P, NS * tf], F32, tag="arena")
    st = lambda i: arena[:, i * tf : (i + 1) * tf]
    sum_r, sum_a, mx, mn, xr = st(0), st(1), st(2), st(3), st(4)
    p0, p1, cc, gam, bet = st(6), st(7), st(12), st(13), st(14)

    nc.vector.tensor_reduce(sum_r, rr3, axis=X, op=op.add)
    nc.vector.tensor_reduce(sum_a, aa3, axis=X, op=op.add)
    nc.vector.tensor_reduce(mx, aa3, axis=X, op=op.max)
    E("mn").tensor_reduce(mn, aa3, axis=X, op=op.min)
    nc.vector.tensor_reduce(xr.bitcast(U32), dd_u3, axis=X, op=op.bitwise_xor)

    # ---- per-point smalls ----
    # parity0: ps = 2*round(sum_r/2) - sum_r in {-1,0,1}; p0 = ps^2 in {0,1}
    h = st(8)
    nc.vector.tensor_scalar(h, sum_r, 0.5, MAGIC, op0=op.mult, op1=op.add)
    nc.vector.tensor_scalar(h, h, MAGIC, None, op0=op.subtract)
    ps = st(9)
    nc.vector.scalar_tensor_tensor(ps, h, 2.0, sum_r, op0=op.mult, op1=op.subtract)
    nc.vector.tensor_tensor(p0, ps, ps, op.mult)
    # parity1 = p0 XOR sign-parity: npf = float(xr >> 31)
    nbu = st(5)
    _ts_u32_single(nc.vector, nbu.bitcast(U32), xr.bitcast(U32), 31,
                   op.logical_shift_right)
    npf = st(11)
    nc.vector.tensor_copy(npf, nbu.bitcast(U32))
    nc.vector.tensor_tensor(p1, p0, npf, op.not_equal)

    # compare: c = [sum_a - 2*p1*mn <= 2*p0*mx - p0 + 2]
    m0, m1 = st(10), st(11)  # m1 overwrites npf (consumed)
    nc.vector.tensor_tensor(m0, p0, mx, op.mult)
    nc.vector.tensor_tensor(m1, p1, mn, op.mult)
    lhs = st(8)  # h slot free
    nc.vector.scalar_tensor_tensor(lhs, m1, -2.0, sum_a, op0=op.mult, op1=op.add)
    rhs = st(5)  # nbu slot free
    nc.vector.scalar_tensor_tensor(rhs, m0, 2.0, p0, op0=op.mult, op1=op.subtract)
    nc.vector.tensor_scalar(rhs, rhs, 2.0, None, op0=op.add)
    nc.vector.tensor_tensor(cc, lhs, rhs, op.is_le)
    cc_u = cc.bitcast(U32)

    # gates: gamma = 2c-1; beta' = 0.5-0.5c; v* = c ? (m0+p0-1) : (m1+p1-1)
    nc.vector.tensor_scalar(gam, cc, 2.0, 1.0, op0=op.mult, op1=op.subtract)
    nc.vector.tensor_scalar(bet, cc, -0.5, 0.5, op0=op.mult, op1=op.add)
    w0, w1 = st(8), st(5)
    nc.vector.scalar_tensor_tensor(w0, p0, -1.0, m0, op0=op.add, op1=op.add)
    nc.vector.scalar_tensor_tensor(w1, p1, -1.0, m1, op0=op.add, op1=op.add)
    vstar = st(9)  # ps slot free
    nc.vector.tensor_copy(vstar, w1)
    nc.vector.copy_predicated(vstar, cc_u, w0)

    # ---- gate + assembly ----
    vb = vstar.unsqueeze(2).broadcast_to([P, tf, 8])
    ee = work.tile([P, FE], F32, tag="ee")
    ee3 = ee[:].rearrange("p (t c) -> p t c", c=8)
    E("e").tensor_tensor(ee3, aa3, vb, op.is_equal)

    # sigma = s*gamma in bit domain: (d0 & 0x80000000) XOR gamma_b
    sg = work.tile([P, FE], F32, tag="sg")
    sg_u3 = sg[:].bitcast(U32).rearrange("p (t c) -> p t c", c=8)
    gam_ub = gam.bitcast(U32).unsqueeze(2).broadcast_to([P, tf, 8])
    _stt_u32(nc.vector, sg_u3, dd_u3, 0x80000000, gam_ub, op.bitwise_and, op.bitwise_xor)

    # q = e - beta'_b ; ts = sigma * q
    bet_b = bet.unsqueeze(2).broadcast_to([P, tf, 8])
    qq = work.tile([P, FE], F32, tag="qq")
    qq3 = qq[:].rearrange("p (t c) -> p t c", c=8)
    E("q").tensor_tensor(qq3, ee3, bet_b, op.subtract)
    ts = work.tile([P, FE], F32, tag="ee")  # alias e's slot (e consumed by q)
    nc.vector.tensor_tensor(ts[:], sg[:], qq[:], op.mult)

    # y = r0 + ts
    if ENGINES["pe_y"]:
        psum_pool, ident = pools[3]
        yp = psum_pool.tile([P, FE], F32, tag="yp")
        NCH = 512
        for c0 in range(0, FE, NCH):
            sl = slice(c0, c0 + NCH)
            nc.tensor.matmul(yp[:, sl], ident[:], rr[:, sl], start=True, stop=False)
            nc.tensor.matmul(yp[:, sl], ident[:], ts[:, sl], start=False, stop=True)
        yt = stream.tile([P, FE], F32, tag="yt")
        nc.scalar.copy(yt[:], yp[:])
        nc.sync.dma_start(out=y_rows, in_=yt[:])
    else:
        yt = stream.tile([P, FE], F32, tag="yt")
        nc.vector.tensor_tensor(yt[:], rr[:], ts[:], op.add)
        nc.sync.dma_start(out=y_rows, in_=yt[:])


def build_nc(shard=SHARD, tf=256):
    P = 128
    pts = P * tf
    assert shard % pts == 0
    ntiles = shard // pts

    nc = bacc.Bacc("TRN2", target_bir_lowering=False, debug=False, num_devices=N_CORES)
    xd = nc.declare_dram_parameter("x", [shard, 8], F32, isOutput=False)
    yd = nc.declare_dram_parameter("y", [shard, 8], F32, isOutput=True)

    from concourse.masks import make_identity
    with TileContext(nc) as tc:
        with (
            tc.tile_pool(name="stream", bufs=2) as stream,
            tc.tile_pool(name="work", bufs=2) as work,
            tc.tile_pool(name="small", bufs=2) as small,
            tc.tile_pool(name="const", bufs=1) as cpool,
            tc.tile_pool(name="psum", bufs=2, space="PSUM") as psum_pool,
        ):
            pe = None
            if ENGINES["pe_y"]:
                ident = cpool.tile([P, P], F32, tag="ident")
                make_identity(nc, ident[:])
                pe = (psum_pool, ident)
            for t in range(ntiles):
                _emit_tile(nc, (stream, work, small, pe), xd, yd, t, tf)
    nc.finalize()
    return nc


_BUILD_CACHE = {}


def _get_nc(shard, tf):
    key = (shard, tf)
    if key not in _BUILD_CACHE:
        _BUILD_CACHE[key] = build_nc(shard, tf)
    return _BUILD_CACHE[key]


def kernel(x: np.ndarray) -> np.ndarray:
    x = np.ascontiguousarray(x, dtype=np.float32)
    n = x.shape[0]
    shard = n // N_CORES
    tf = 256
    while shard % (128 * tf) != 0:
        tf //= 2
    nc = _get_nc(shard, tf)
    in_maps = [{"x": x[i * shard : (i + 1) * shard]} for i in range(N_CORES)]
    res = run_bass_kernel_spmd(nc, in_maps, list(range(N_CORES))).results
    return np.concatenate([res[i]["y"] for i in range(N_CORES)], axis=0)


# revision 11
# speedup vs baseline: 2.1965x; 2.1965x over previous
"""Nearest-E8-lattice quantizer (CachedE8Quantizer) as a Bass/Tile kernel on 8 trn2 cores.

Input x: [8388608, 8] fp32. Output: nearest point of E8 = D8 u (D8 + 1/2).

Sharding: data-parallel over the points dim, 1/8 per core (no comms).

Algorithm (per 8-vector x), branch0 = D8, branch1 = D8 + 1/2:
  r0 = round(x) (RNE), d0 = x - r0 (exact), a = |d0|
  The two nearest half-grid points per coord are r0 and r1h = r0 + 0.5*s
  (s = sign(d0)), with |d1| = 0.5 - a. Hence:
    S2_1      = S2_0 - sum(a) + 2
    max|d1|   = 0.5 - min(a);  argmax|d1| = argmin(a)
    parity1   = parity0 XOR parity(#negative d0)   (since f1 = r0 - 1{d0<0})
  Candidate distances with the parity-repair term (flip the worst coord):
    D0 = S2 + p0*(1 - 2*max(a));  D1 = S2 - sum(a) + 2 + p1*(2*min(a))
    c = D0 <= D1  <=>  sum(a) - 2*p1*mn <= 2*p0*mx - p0 + 2
  Output:  y = r0 + s*(beta' + gamma*e),  gamma = 2c-1, beta' = 0.5*(1-c),
    e = 1{a == v*}, v* = gated flip target (mx if c&p0, mn if ~c&p1, else -1).
  s*(...) is applied in bit domain: sigma = (d0 & 0x80000000) XOR gamma.

Per-core per-tile engine split (tf points/partition, FE = 8*tf elems):
  DVE: r0 (TS 2x), 4-5 segmented reduces (1x), gate is_equal, sigma, ts, smalls
  ACT: a = Abs(d0), p0 = Square(ps), beta', PSUM->SBUF y copy
  GPSIMD: d0 = x - r0, min-reduce, q = e - beta'
  PE:  y = I*r0 + I*ts accumulated in PSUM
"""

import numpy as np

from concourse import bacc
import concourse.mybir as mybir
from concourse.alu_op_type import AluOpType as op
from concourse.bass_utils import run_bass_kernel_spmd
from concourse.tile import TileContext

N_POINTS = 8388608
N_CORES = 8
SHARD = N_POINTS // N_CORES  # 1048576 points per core

MAGIC = 12582912.0  # 1.5 * 2**23: (x + MAGIC) - MAGIC == round-half-even(x)
F32 = mybir.dt.float32
I32 = mybir.dt.int32
U32 = mybir.dt.uint32
X = mybir.AxisListType.X
CP = mybir.ActivationFunctionType.Copy

ENGINES = {
    "d0": "vector",      # x - r0
    "mn": "vector",      # min-reduce engine
    "q": "vector",       # q = e - beta'_b
    "ts": "gpsimd",      # ts = sigma * q
    "e": "vector",       # gate equality
    "pe_y": True,        # final add on PE (else DVE)
    "smalls_act": True,  # beta'/p0/npf-cast on ACT
    "skip": (),          # diagnostic: drop op groups ("reds","smalls","gate")
}


def _stt_u32(eng, out, in0, scalar_int, in1, op0, op1):
    """scalar_tensor_tensor with a uint32 immediate."""
    return eng.add_instruction(
        mybir.InstTensorScalarPtr(
            name=eng.bass.get_next_instruction_name(),
            is_scalar_tensor_tensor=True,
            op0=op0,
            op1=op1,
            ins=[
                eng.lower_ap(in0),
                mybir.ImmediateValue(dtype=U32, value=scalar_int),
                eng.lower_ap(in1),
            ],
            outs=[eng.lower_ap(out)],
        )
    )


def _ts_u32_single(eng, out, in0, scalar_int, op0):
    """tensor_scalar with a uint32 immediate (single op)."""
    return eng.add_instruction(
        mybir.InstTensorScalarPtr(
            name=eng.bass.get_next_instruction_name(),
            op0=op0,
            ins=[
                eng.lower_ap(in0),
                mybir.ImmediateValue(dtype=U32, value=scalar_int),
            ],
            outs=[eng.lower_ap(out)],
        )
    )


def _emit_tile(nc, pools, xd, yd, t, tf):
    E = lambda k: getattr(nc, ENGINES[k])
    P = 128
    pts = P * tf
    FE = tf * 8
    stream, work, small = pools[:3]

    s0 = t * pts
    x_rows = xd[s0 : s0 + pts, :].rearrange("(p f) c -> p (f c)", p=P)
    y_rows = yd[s0 : s0 + pts, :].rearrange("(p f) c -> p (f c)", p=P)

    xt = stream.tile([P, FE], F32, tag="xt")
    nc.sync.dma_start(out=xt[:], in_=x_rows)

    # r0 = round(x)  (RNE via magic number), DVE TS 2x
    rr = work.tile([P, FE], F32, tag="rr")
    nc.vector.tensor_scalar(rr[:], xt[:], MAGIC, MAGIC, op0=op.add, op1=op.subtract)

    # d0 = x - r0 (exact)
    dd = work.tile([P, FE], F32, tag="dd")
    E("d0").tensor_tensor(dd[:], xt[:], rr[:], op.subtract)
    dd_u3 = dd[:].bitcast(U32).rearrange("p (t c) -> p t c", c=8)

    # a = |d0| on ACT
    aa = work.tile([P, FE], F32, tag="aa")
    nc.scalar.activation(aa[:], dd[:], mybir.ActivationFunctionType.Abs)
    aa3 = aa[:].rearrange("p (t c) -> p t c", c=8)
    rr3 = rr[:].rearrange("p (t c) -> p t c", c=8)

    # ---- segmented reduces -> per-point stats [P, tf] ----
    NS = 15
    arena = small.tile([P, NS * tf], F32, tag="arena")
    st = lambda i: arena[:, i * tf : (i + 1) * tf]
    sum_r, sum_a, mx, mn, xr = st(0), st(1), st(2), st(3), st(4)
    p0, p1, cc, gam, bet = st(6), st(7), st(12), st(13), st(14)

    nc.vector.tensor_reduce(sum_r, rr3, axis=X, op=op.add)
    nc.vector.tensor_reduce(sum_a, aa3, axis=X, op=op.add)
    nc.vector.tensor_reduce(mx, aa3, axis=X, op=op.max)
    E("mn").tensor_reduce(mn, aa3, axis=X, op=op.min)
    nc.vector.tensor_reduce(xr.bitcast(U32), dd_u3, axis=X, op=op.bitwise_xor)

    # ---- per-point smalls ----
    # parity0: ps = 2*round(sum_r/2) - sum_r in {-1,0,1}; p0 = ps^2 in {0,1}
    h = st(8)
    nc.vector.tensor_scalar(h, sum_r, 0.5, MAGIC, op0=op.mult, op1=op.add)
    nc.vector.tensor_scalar(h, h, MAGIC, None, op0=op.subtract)
    ps = st(9)
    nc.vector.scalar_tensor_tensor(ps, h, 2.0, sum_r, op0=op.mult, op1=op.subtract)
    if ENGINES["smalls_act"]:
        nc.scalar.square(p0, ps)
    else:
        nc.vector.tensor_tensor(p0, ps, ps, op.mult)
    # parity1 = p0 XOR sign-parity: npf = float(xr >> 31)
    nbu = st(5)
    _ts_u32_single(nc.vector, nbu.bitcast(U32), xr.bitcast(U32), 31,
                   op.logical_shift_right)
    npf = st(11)
    if ENGINES["smalls_act"]:
        nc.scalar.copy(npf, nbu.bitcast(U32))
    else:
        nc.vector.tensor_copy(npf, nbu.bitcast(U32))
    nc.vector.tensor_tensor(p1, p0, npf, op.not_equal)

    # compare: c = [sum_a - 2*p1*mn <= 2*p0*mx - p0 + 2]
    m0, m1 = st(10), st(11)  # m1 overwrites npf (consumed)
    nc.vector.tensor_tensor(m0, p0, mx, op.mult)
    nc.vector.tensor_tensor(m1, p1, mn, op.mult)
    lhs = st(8)  # h slot free
    nc.vector.scalar_tensor_tensor(lhs, m1, -2.0, sum_a, op0=op.mult, op1=op.add)
    rhs = st(5)  # nbu slot free
    nc.vector.scalar_tensor_tensor(rhs, m0, 2.0, p0, op0=op.mult, op1=op.subtract)
    nc.vector.tensor_scalar(rhs, rhs, 2.0, None, op0=op.add)
    nc.vector.tensor_tensor(cc, lhs, rhs, op.is_le)
    cc_u = cc.bitcast(U32)

    # gates: gamma = 2c-1; beta' = 0.5-0.5c; v* = c ? (m0+p0-1) : (m1+p1-1)
    nc.vector.tensor_scalar(gam, cc, 2.0, 1.0, op0=op.mult, op1=op.subtract)
    if ENGINES["smalls_act"]:
        nc.scalar.activation(bet, cc, CP, bias=0.5, scale=-0.5)
    else:
        nc.vector.tensor_scalar(bet, cc, -0.5, 0.5, op0=op.mult, op1=op.add)
    w0, vstar = st(8), st(9)  # ps slot free
    nc.vector.scalar_tensor_tensor(w0, p0, -1.0, m0, op0=op.add, op1=op.add)
    nc.vector.scalar_tensor_tensor(vstar, p1, -1.0, m1, op0=op.add, op1=op.add)
    nc.vector.copy_predicated(vstar, cc_u, w0)

    # ---- gate + assembly ----
    vb = vstar.unsqueeze(2).broadcast_to([P, tf, 8])
    ee = work.tile([P, FE], F32, tag="ee")
    ee3 = ee[:].rearrange("p (t c) -> p t c", c=8)
    E("e").tensor_tensor(ee3, aa3, vb, op.is_equal)

    # sigma = s*gamma in bit domain: (d0 & 0x80000000) XOR gamma_b
    sg = work.tile([P, FE], F32, tag="sg")
    sg_u3 = sg[:].bitcast(U32).rearrange("p (t c) -> p t c", c=8)
    gam_ub = gam.bitcast(U32).unsqueeze(2).broadcast_to([P, tf, 8])
    _stt_u32(nc.vector, sg_u3, dd_u3, 0x80000000, gam_ub, op.bitwise_and, op.bitwise_xor)

    # q = e - beta'_b ; ts = sigma * q
    bet_b = bet.unsqueeze(2).broadcast_to([P, tf, 8])
    qq = work.tile([P, FE], F32, tag="qq")
    qq3 = qq[:].rearrange("p (t c) -> p t c", c=8)
    E("q").tensor_tensor(qq3, ee3, bet_b, op.subtract)
    ts = work.tile([P, FE], F32, tag="ee")  # alias e's slot (e consumed by q)
    E("ts").tensor_tensor(ts[:], sg[:], qq[:], op.mult)

    # y = r0 + ts
    if ENGINES["pe_y"]:
        psum_pool, ident = pools[3]
        yp = psum_pool.tile([P, FE], F32, tag="yp")
        NCH = 512
        for c0 in range(0, FE, NCH):
            sl = slice(c0, c0 + NCH)
            nc.tensor.matmul(yp[:, sl], ident[:], rr[:, sl], start=True, stop=False)
            nc.tensor.matmul(yp[:, sl], ident[:], ts[:, sl], start=False, stop=True)
        yt = stream.tile([P, FE], F32, tag="yt")
        nc.scalar.copy(yt[:], yp[:])
        nc.sync.dma_start(out=y_rows, in_=yt[:])
    else:
        yt = stream.tile([P, FE], F32, tag="yt")
        nc.vector.tensor_tensor(yt[:], rr[:], ts[:], op.add)
        nc.sync.dma_start(out=y_rows, in_=yt[:])


def build_nc(shard=SHARD, tf=256):
    P = 128
    pts = P * tf
    assert shard % pts == 0
    ntiles = shard // pts

    nc = bacc.Bacc("TRN2", target_bir_lowering=False, debug=False, num_devices=N_CORES)
    xd = nc.declare_dram_parameter("x", [shard, 8], F32, isOutput=False)
    yd = nc.declare_dram_parameter("y", [shard, 8], F32, isOutput=True)

    from concourse.masks import make_identity
    with TileContext(nc) as tc:
        with (
            tc.tile_pool(name="stream", bufs=2) as stream,
            tc.tile_pool(name="work", bufs=2) as work,
            tc.tile_pool(name="small", bufs=2) as small,
            tc.tile_pool(name="const", bufs=1) as cpool,
            tc.tile_pool(name="psum", bufs=2, space="PSUM") as psum_pool,
        ):
            pe = None
            if ENGINES["pe_y"]:
                ident = cpool.tile([P, P], F32, tag="ident")
                make_identity(nc, ident[:])
                pe = (psum_pool, ident)
            for t in range(ntiles):
                _emit_tile(nc, (stream, work, small, pe), xd, yd, t, tf)
    nc.finalize()
    return nc


_BUILD_CACHE = {}


def _get_nc(shard, tf):
    key = (shard, tf)
    if key not in _BUILD_CACHE:
        _BUILD_CACHE[key] = build_nc(shard, tf)
    return _BUILD_CACHE[key]


def kernel(x: np.ndarray) -> np.ndarray:
    x = np.ascontiguousarray(x, dtype=np.float32)
    n = x.shape[0]
    shard = n // N_CORES
    tf = 256
    while shard % (128 * tf) != 0:
        tf //= 2
    nc = _get_nc(shard, tf)
    in_maps = [{"x": x[i * shard : (i + 1) * shard]} for i in range(N_CORES)]
    res = run_bass_kernel_spmd(nc, in_maps, list(range(N_CORES))).results
    return np.concatenate([res[i]["y"] for i in range(N_CORES)], axis=0)
